# revision 49
# baseline (speedup 1.0000x reference)
"""Trainium2 Bass kernel for nn_Block_48610439856264 (DiT-style transformer block).

B=4, N=2048, C=512, H=8 heads, D=64, d_ff=2048, fp32 I/O.

Sharding: 8 cores = 4 batches x 2 token-halves. Each core receives the full
batch (own token half first) so k/v/s are computed locally over all 2048
tokens (duplicated across the 2 cores of a batch; no collectives), while
q/gate/proj/MLP/output cover only the core's own 1024 tokens.

Device pipeline per core:
  LN1(x), LN2(e) token-major (bn_stats; batched group DMAs since descriptor
  issue on SP is serial) -> bf16 -> xbar transpose to c-major (zx) / DRAM
  roundtrip transpose (ze)
  kk^T = w_k^T zx^T + w_s^T ze^T (PSUM-accumulated), q^T, gate^T (c-major);
  e-independent projections are ordered ahead of kk so the in-order PE never
  head-of-line blocks on the ze roundtrip
  v token-major (lhsT = zx^T blocks), augmented with a ones column per head
  attention runs chunk-major over two 512-token nq chunks; per (pair, tile,
  chunk): one 2-bank PSUM scores tile [A|B], ONE exp on ACT -> E bf16;
  attn@v accumulates per-chunk 1-bank tiles; lag deque keeps exp (the pacer)
  decoupled from attn@v
  chunk-0 proj + LN3 (ln/exp rstd, DRAM-transposed z3) + raw fc1 are emitted
  one unit per attention entry underneath chunk-1's exp-bound stream; MLP
  weights and the x residual reload are prefetched under attention
  tail: chunk-1 proj/LN3, single gelu-table switch, gelu+fc2 interleaved
"""

import numpy as np
import ml_dtypes

N_CORES = 8
B, N, C = 4, 2048, 512
H, D = 8, 64
DFF = 4 * C
P = 128
NT = N // P          # 16 full-token tiles
NTO = NT // 2        # 8 own-token tiles
CT = C // P          # 4 channel tiles
DFT = DFF // P       # 16 d_ff tiles
TOK_OWN = N // 2     # 1024
EPS = 1e-5
NCH_FULL = N // 512      # 4 chunks of 512 tokens
NCH_OWN = TOK_OWN // 512  # 2

_CACHE = {}


def _build_nc():
    import concourse.bacc as bacc
    import concourse.mybir as mybir
    import concourse.tile as tile

    FP32 = mybir.dt.float32
    BF16 = mybir.dt.bfloat16

    nc = bacc.Bacc("TRN2", num_devices=N_CORES)

    # ---- DRAM I/O ----
    xb_d = nc.dram_tensor("xb", [N, C], FP32, kind="ExternalInput").ap()
    eb_d = nc.dram_tensor("eb", [N, C], FP32, kind="ExternalInput").ap()
    wqkv_d = nc.dram_tensor("wqkv", [C, 3 * C], BF16, kind="ExternalInput").ap()
    ws_d = nc.dram_tensor("ws", [C, C], BF16, kind="ExternalInput").ap()
    wgate_d = nc.dram_tensor("wgate", [C, C], BF16, kind="ExternalInput").ap()
    wproj_d = nc.dram_tensor("wproj", [C, C], BF16, kind="ExternalInput").ap()
    wfc1_d = nc.dram_tensor("wfc1", [C, DFF], BF16, kind="ExternalInput").ap()
    wfc2_d = nc.dram_tensor("wfc2", [DFF, C], BF16, kind="ExternalInput").ap()
    out_d = nc.dram_tensor("out", [TOK_OWN, C], FP32, kind="ExternalOutput").ap()

    with tile.TileContext(nc) as tc:
        _build_body(nc, tc, mybir,
                    xb_d, eb_d, out_d,
                    wqkv_d, ws_d, wgate_d, wproj_d, wfc1_d, wfc2_d)

    nc.compile()
    return nc


def _build_body(nc, tc, mybir,
                xb_r_, eb_r_, out_d,
                wqkv_d, ws_d, wgate_d, wproj_d, wfc1_d, wfc2_d):
    from contextlib import ExitStack
    from concourse.masks import make_identity

    FP32 = mybir.dt.float32
    BF16 = mybir.dt.bfloat16
    Act = mybir.ActivationFunctionType
    Alu = mybir.AluOpType

    xb_r = xb_r_.rearrange("(t p) c -> t p c", p=P)
    eb_r = eb_r_.rearrange("(t p) c -> t p c", p=P)
    out_r = out_d.rearrange("(t p) c -> t p c", p=P)

    L0 = ExitStack()
    with L0:
        consts = L0.enter_context(tc.tile_pool(name="consts", bufs=1))
        stat_pool = L0.enter_context(tc.tile_pool(name="stats", bufs=6))
        z_pool = L0.enter_context(tc.tile_pool(name="zp", bufs=2))
        zg_pool = L0.enter_context(tc.tile_pool(name="zg", bufs=1))
        es_x = ExitStack()  # x own-half tiles: freed after LN1, reloaded at proj
        # long-lived attention inputs/outputs
        kkT = [L0.enter_context(tc.tile_pool(name=f"kkT{i}", bufs=1))
               .tile([P, N], BF16, name=f"kkTt{i}") for i in range(CT)]
        qT = [L0.enter_context(tc.tile_pool(name=f"qT{i}", bufs=1))
              .tile([P, TOK_OWN], BF16, name=f"qTt{i}") for i in range(CT)]
        gTh = L0.enter_context(tc.tile_pool(name="gTh", bufs=1)) \
            .tile([D, H, TOK_OWN], BF16, name="gTht")
        vpool = L0.enter_context(tc.tile_pool(name="vp", bufs=NT))
        VPAD = (H - 1) * (D + 1) + P  # pad: last lhsT window ends at 7*65+128
        v_aug = [vpool.tile([P, VPAD], BF16, name=f"vaug{i}", tag="vaug")
                 for i in range(NT)]
        ogT = [L0.enter_context(tc.tile_pool(name=f"ogT{i}", bufs=1))
               .tile([P, TOK_OWN], BF16, name=f"ogTt{i}") for i in range(CT)]
        dram_p = L0.enter_context(tc.tile_pool(name="zdram", bufs=1, space="DRAM"))
        # scores PSUM on the right side: coexists with psC (left) during the
        # projection phase so pair-0 exp can start early.
        psS = L0.enter_context(
            tc.tile_pool(name="psS", bufs=2, space="PSUM", side="right"))

        eps_sb = consts.tile([P, 1], FP32)
        nc.vector.memset(eps_sb[:], EPS)
        ident = consts.tile([P, P], BF16)
        make_identity(nc, ident[:])

        def ln_group(tiles, mvtag, sink, t0, lnexp=False, copy_eng=None):
            g = len(tiles)
            mv = stat_pool.tile([P, g, 2], FP32, name=f"mv_{mvtag}", tag="mv")
            st6 = stat_pool.tile([P, 6], FP32, name=f"st6_{mvtag}", tag="st6")
            for i, xt in enumerate(tiles):
                nc.vector.bn_stats(st6[:], xt[:])
                nc.vector.bn_aggr(mv[:, i, :], st6[:])
            rstd = stat_pool.tile([P, g], FP32, name=f"rstd_{mvtag}", tag="rstd")
            if lnexp:
                # rstd = exp(-0.5*ln(var+eps)): stays inside the ln/exp ACT
                # table, so LN3 can run mid-exp-stream with no table switch
                lnv = stat_pool.tile([P, g], FP32, name=f"lnv_{mvtag}", tag="sd")
                nc.scalar.activation(lnv[:], mv[:, :, 1], Act.Ln, bias=eps_sb[:])
                nc.scalar.activation(rstd[:], lnv[:], Act.Exp, scale=-0.5)
            else:
                sd = stat_pool.tile([P, g], FP32, name=f"sd_{mvtag}", tag="sd")
                nc.scalar.activation(sd[:], mv[:, :, 1], Act.Sqrt, bias=eps_sb[:])
                nc.vector.reciprocal_approx_fast(rstd[:], sd[:])
            # LN apply runs on ACT (Identity is in every act table):
            # z = (x - mu)*rstd = Identity(x*rstd + (-mu*rstd))
            nmr = stat_pool.tile([P, g], FP32, name=f"nmr_{mvtag}", tag="nmr")
            nc.vector.scalar_tensor_tensor(
                nmr[:], mv[:, :, 0], -1.0, rstd[:], Alu.mult, Alu.mult)
            mode, dest = sink
            if mode == "dram":
                zg = zg_pool.tile([P, g, C], BF16, name=f"zg_{mvtag}", tag="zg")
                for i, xt in enumerate(tiles):
                    nc.scalar.activation(
                        zg[:, i, :], xt[:], Act.Identity,
                        bias=nmr[:, i : i + 1], scale=rstd[:, i : i + 1])
                nc.sync.dma_start(dest[t0 // 4], zg[:])
            else:
                T_all, pspool, pstag = dest
                for i, xt in enumerate(tiles):
                    t = t0 + i
                    zt = z_pool.tile([P, C], BF16, name=f"z_{mvtag}_{i}", tag="z")
                    nc.scalar.activation(
                        zt[:], xt[:], Act.Identity,
                        bias=nmr[:, i : i + 1], scale=rstd[:, i : i + 1])
                    for c in range(CT):
                        pt = pspool.tile([P, P], BF16, name=f"pt{c}", tag=pstag)
                        nc.tensor.transpose(
                            pt[:], zt[:, c * P : (c + 1) * P], ident[:]
                        )
                        if copy_eng is None:
                            nc.vector.tensor_copy(
                                T_all[:, c, t * P : (t + 1) * P], pt[:]
                            )
                        else:
                            copy_eng.copy(
                                T_all[:, c, t * P : (t + 1) * P], pt[:]
                            )

        def transpose_in(zdram, T_tiles, ntok):
            for c in range(CT):
                nc.sync.dma_start(
                    T_tiles[c][:, 0:ntok],
                    zdram[:, c * P : (c + 1) * P],
                    transpose=True,
                )

        es_att = ExitStack()  # epool/rpool: right side, closed before MLP
        L2 = ExitStack()      # left-side projection-phase allocations
        try:
            zxT_p = L2.enter_context(tc.tile_pool(name="zxTp", bufs=1))
            zxT = zxT_p.tile([P, CT, N], BF16, name="zxTall", tag="zxT")
            wB = L2.enter_context(tc.tile_pool(name="wB", bufs=1))
            psC = L2.enter_context(tc.tile_pool(name="psC", bufs=2, space="PSUM"))
            L2a = L2.enter_context(ExitStack())
            zeT_p = L2a.enter_context(tc.tile_pool(name="zeTp", bufs=CT))
            zeT = [zeT_p.tile([P, N], BF16, name=f"zeT{i}", tag="zeT")
                   for i in range(CT)]
            wsp = L2a.enter_context(tc.tile_pool(name="wsp", bufs=1))
            xo_pool = es_x.enter_context(tc.tile_pool(name="xo", bufs=2))

            ze_dram = dram_p.tile([N, C], BF16, name="ze_dram")
            ze_dram_g = ze_dram[:].rearrange("(g t p) c -> g p t c", p=P, t=4)

            epool = es_att.enter_context(
                tc.tile_pool(name="epool", bufs=17, side="right"))
            E0 = {}

            def transpose_in_half(zdram, T_tiles, half):
                for c in range(CT):
                    nc.sync.dma_start(
                        T_tiles[c][:, half * TOK_OWN : (half + 1) * TOK_OWN],
                        zdram[half * TOK_OWN : (half + 1) * TOK_OWN,
                              c * P : (c + 1) * P],
                        transpose=True,
                    )

            # ---------- projections + early pair-0 scores ----------
            def q_proj(m):
                pq = psC.tile([P, TOK_OWN], FP32, name=f"pq{m}", tag="pc")
                for k in range(CT):
                    lw = wqkv_sb[:, k, m * P : (m + 1) * P]
                    for ch in range(NCH_OWN):
                        nc.tensor.matmul(
                            pq[:, ch * 512 : (ch + 1) * 512], lw,
                            zxT[:, k, ch * 512 : (ch + 1) * 512],
                            start=(k == 0), stop=(k == CT - 1),
                        )
                nc.vector.tensor_copy(qT[m][:], pq[:])

            def kk_half(m, half):
                # chunks (2*half, 2*half+1) of kk^T row-tile m
                pc = psC.tile([P, TOK_OWN], FP32, name=f"pc{m}_{half}", tag="pc")
                for k in range(CT):
                    lw = wqkv_sb[:, k, C + m * P : C + (m + 1) * P]
                    for i in range(2):
                        ch = 2 * half + i
                        nc.tensor.matmul(
                            pc[:, i * 512 : (i + 1) * 512], lw,
                            zxT[:, k, ch * 512 : (ch + 1) * 512],
                            start=(k == 0), stop=False,
                        )
                for k in range(CT):
                    lw = ws_sb[:, k, m * P : (m + 1) * P]
                    for i in range(2):
                        ch = 2 * half + i
                        nc.tensor.matmul(
                            pc[:, i * 512 : (i + 1) * 512], lw,
                            zeT[k][:, ch * 512 : (ch + 1) * 512],
                            start=False, stop=(k == CT - 1),
                        )
                nc.vector.tensor_copy(
                    kkT[m][:, half * TOK_OWN : (half + 1) * TOK_OWN], pc[:])

            def scores_exp(pr, t, ch):
                # heads A|B side by side: one 2-bank PSUM tile, one exp
                sc = psS.tile([P, 2, 512], FP32, name="scAB", tag="sc")
                nc.tensor.matmul(
                    sc[:, 0, :],
                    kkT[pr][0:D, t * P : (t + 1) * P],
                    qT[pr][0:D, ch * 512 : (ch + 1) * 512],
                    start=True, stop=True,
                )
                nc.tensor.matmul(
                    sc[:, 1, :],
                    kkT[pr][D : 2 * D, t * P : (t + 1) * P],
                    qT[pr][D : 2 * D, ch * 512 : (ch + 1) * 512],
                    start=True, stop=True,
                )
                Et = epool.tile([P, 2 * 512], BF16, name="Et", tag="E")
                nc.scalar.activation(Et[:], sc[:], Act.Exp)
                return Et

            def v_pair(tp):
                # token tiles 2*tp, 2*tp+1
                pv = psC.tile([P, TOK_OWN], FP32, name=f"pv{tp}", tag="pc")
                for k in range(CT):
                    for i in range(2):
                        t = 2 * tp + i
                        nc.tensor.matmul(
                            pv[:, i * 512 : (i + 1) * 512],
                            zxT[:, k, t * P : (t + 1) * P],
                            wqkv_sb[:, k, 2 * C : 3 * C],
                            start=(k == 0), stop=(k == CT - 1),
                        )
                for i in range(2):
                    t = 2 * tp + i
                    nc.gpsimd.memset(v_aug[t][:, H * (D + 1) : VPAD], 0.0)
                    va = v_aug[t][:, 0 : H * (D + 1)].rearrange(
                        "p (h x) -> p h x", x=D + 1)
                    nc.scalar.copy(
                        va[:, :, 0:D],
                        pv[:, i * 512 : (i + 1) * 512].rearrange(
                            "p (h d) -> p h d", d=D),
                    )
                    nc.gpsimd.memset(va[:, :, D : D + 1], 1.0)

            def gate_proj(m):
                pg = psC.tile([P, TOK_OWN], FP32, name=f"pg{m}", tag="pc")
                for k in range(CT):
                    lw = wgate_sb[:, k, m * P : (m + 1) * P]
                    for ch in range(NCH_OWN):
                        nc.tensor.matmul(
                            pg[:, ch * 512 : (ch + 1) * 512], lw,
                            zxT[:, k, ch * 512 : (ch + 1) * 512],
                            start=(k == 0), stop=(k == CT - 1),
                        )
                nc.scalar.copy(gTh[0:D, 2 * m, :], pg[0:D, :])
                nc.scalar.copy(gTh[0:D, 2 * m + 1, :], pg[D : 2 * D, :])

            with (
                tc.tile_pool(name="xrp", bufs=2) as xr_pool,
                tc.tile_pool(name="ep", bufs=2) as e_pool,
            ):
                # one DMA descriptor per 4-tile group: descriptor issue on the
                # SP sequencer is serial (~0.7us each) and gates startup
                xb_g = xb_r_.rearrange("(g t p) c -> g p t c", p=P, t=4)
                eb_g = eb_r_.rearrange("(g t p) c -> g p t c", p=P, t=4)
                xg_tiles = []
                for g in range(2):
                    xt = xo_pool.tile([P, 4, C], FP32, name=f"xg{g}", tag="xg")
                    nc.sync.dma_start(xt[:], xb_g[g])
                    xg_tiles.append(xt)
                wqkv_sb = wB.tile([P, CT, 3 * C], BF16)
                nc.sync.dma_start(
                    wqkv_sb[:], wqkv_d.rearrange("(k p) n -> p k n", p=P))
                eg_tiles = []
                for g in range(2):
                    et = e_pool.tile([P, 4, C], FP32, name=f"eg{g}", tag="e")
                    nc.sync.dma_start(et[:], eb_g[g])
                    eg_tiles.append(et)
                ws_sb = wsp.tile([P, CT, C], BF16)
                nc.sync.dma_start(
                    ws_sb[:], ws_d.rearrange("(k p) n -> p k n", p=P))
                for g in range(2, 4):
                    xt = xr_pool.tile([P, 4, C], FP32, name=f"xg{g}", tag="xr")
                    nc.sync.dma_start(xt[:], xb_g[g])
                    xg_tiles.append(xt)
                wgate_sb = wB.tile([P, CT, C], BF16)
                nc.sync.dma_start(
                    wgate_sb[:], wgate_d.rearrange("(k p) n -> p k n", p=P))
                for g in range(2, 4):
                    et = e_pool.tile([P, 4, C], FP32, name=f"eg{g}", tag="e")
                    nc.sync.dma_start(et[:], eb_g[g])
                    eg_tiles.append(et)
                x_all = [xg_tiles[t // 4][:, t % 4, :] for t in range(NT)]
                e_groups = [[eg_tiles[g][:, i, :] for i in range(4)]
                            for g in range(4)]
                x_own = x_all[:NTO]

                # x-side first: q/v fill the PE while the e-side LN -> DRAM
                # -> transpose roundtrip completes (PE is in-order, so kk must
                # come after enough e-independent work)
                ln_group(x_all[0:4], "x0", ("pe", (zxT, psC, "pc")), 0)
                ln_group(x_all[4:8], "x1", ("pe", (zxT, psC, "pc")), 4)
                q_proj(0)
                ln_group(e_groups[0], "e0", ("dram", ze_dram_g), 0)
                ln_group(e_groups[1], "e1", ("dram", ze_dram_g), 4)
                transpose_in_half(ze_dram[:], zeT, 0)
                v_pair(0)
                v_pair(1)
                ln_group(x_all[8:12], "x2", ("pe", (zxT, psC, "pc")), 8)
                v_pair(2)
                v_pair(3)
                q_proj(1)
                gate_proj(0)
                gate_proj(1)
                kk_half(0, 0)
                E0[0] = scores_exp(0, 0, 0)
                E0[1] = scores_exp(0, 1, 0)
                kk_half(1, 0)
                E0[2] = scores_exp(0, 2, 0)
                E0[3] = scores_exp(0, 3, 0)
                ln_group(e_groups[2], "e2", ("dram", ze_dram_g), 8)
                kk_half(2, 0)
                E0[4] = scores_exp(0, 4, 0)
                E0[5] = scores_exp(0, 5, 0)
                v_pair(4)
                q_proj(2)
                kk_half(3, 0)
                E0[6] = scores_exp(0, 6, 0)
                E0[7] = scores_exp(0, 7, 0)
                ln_group(x_all[12:16], "x3", ("pe", (zxT, psC, "pc")), 12)
                v_pair(5)
                gate_proj(2)
                ln_group(e_groups[3], "e3", ("dram", ze_dram_g), 12)
                transpose_in_half(ze_dram[:], zeT, 1)

            es_x.close()
            N_EARLY = 15
            q_proj(3)
            for tp in range(6, NT // 2):
                v_pair(tp)
            gate_proj(3)
            kk_half(0, 1)
            for tt in range(8, 15):
                E0[tt] = scores_exp(0, tt, 0)
            kk_half(1, 1)
            kk_half(2, 1)
            kk_half(3, 1)
            L2a.close()  # zeT, ws freed
        finally:
            L2.close()  # zxT, wqkv, wgate, psC freed

        rpool = es_att.enter_context(
            tc.tile_pool(name="rpool", bufs=2, side="right"))
        # prefetch MLP weights under the attention stream (L2 SBUF just freed)
        wE = L0.enter_context(tc.tile_pool(name="wE", bufs=1))
        wproj_sb = wE.tile([P, CT, C], BF16)
        nc.sync.dma_start(wproj_sb[:],
                          wproj_d.rearrange("(k p) n -> p k n", p=P))
        wfc1_sb = wE.tile([P, CT, DFF], BF16)
        nc.sync.dma_start(wfc1_sb[:],
                          wfc1_d.rearrange("(k p) n -> p k n", p=P))
        wfc2_sb = wE.tile([P, DFT, C], BF16)
        nc.sync.dma_start(wfc2_sb[:],
                          wfc2_d.rearrange("(k p) n -> p k n", p=P))

        # ---------- attention + chunk-0 MLP fill + tail ----------
        with (
            tc.tile_pool(name="psO", bufs=4, space="PSUM") as psO,
            tc.tile_pool(name="z3Tp", bufs=1) as z3T_pool,
            tc.tile_pool(name="xm", bufs=NTO) as xm_pool,
            tc.tile_pool(name="xrl", bufs=1) as xrl_pool,
            tc.tile_pool(name="hrawp", bufs=1) as hraw_pool,
        ):
            z3T = z3T_pool.tile([P, CT, TOK_OWN], BF16, name="z3Tall",
                                tag="z3Tt")
            hraw = hraw_pool.tile([P, DFT, 512], BF16, name="hraw")
            z3_dram = dram_p.tile([TOK_OWN, C], BF16, name="z3_dram")
            z3_dram_g = z3_dram[:].rearrange("(g t p) c -> g p t c", p=P, t=4)
            xrl0 = xrl_pool.tile([P, 4, C], FP32, name="xrl0", tag="xrl")
            nc.sync.dma_start(xrl0[:], xb_g[0])
            x_own = [None] * NTO
            x_own[0:4] = [xrl0[:, t, :] for t in range(4)]
            xm0 = []

            def transpose_in_z3(ch):
                for c in range(CT):
                    nc.sync.dma_start(
                        z3T[:, c, ch * 512 : (ch + 1) * 512],
                        z3_dram[ch * 512 : (ch + 1) * 512,
                                c * P : (c + 1) * P],
                        transpose=True,
                    )

            def proj_tile(t, xm_sink):
                pp = psO.tile([P, C], FP32, name=f"pp{t}", tag="po")
                for k in range(CT):
                    nc.tensor.matmul(
                        pp[:], ogT[k][:, t * P : (t + 1) * P],
                        wproj_sb[:, k, :],
                        start=(k == 0), stop=(k == CT - 1),
                    )
                xmt = xm_pool.tile([P, C], FP32, name=f"xm{t}", tag="xm")
                nc.vector.tensor_add(xmt[:], x_own[t][:], pp[:])
                xm_sink.append(xmt)

            def ch0_mlp_gen():
                # chunk-0 proj -> LN3 (DRAM-transposed, ln/exp rstd) -> raw
                # fc1 staged to SBUF; one unit emitted per attention entry
                for i in range(4):
                    proj_tile(i, xm0)
                    yield
                ln_group(xm0, "x30", ("dram", z3_dram_g), 0, lnexp=True)
                yield
                transpose_in_z3(0)
                yield
                xrl1 = xrl_pool.tile([P, 4, C], FP32, name="xrl1", tag="xrl")
                nc.sync.dma_start(xrl1[:], xb_g[1])
                x_own[4:8] = [xrl1[:, t, :] for t in range(4)]
                yield
                for _ in range(5):
                    yield  # attention entries cover the z3 DMA roundtrip
                for m in range(DFT):
                    pf = psO.tile([P, 512], FP32, name=f"pf0_{m}", tag="po")
                    for k in range(CT):
                        nc.tensor.matmul(
                            pf[:], wfc1_sb[:, k, m * P : (m + 1) * P],
                            z3T[:, k, 0:512],
                            start=(k == 0), stop=(k == CT - 1),
                        )
                    nc.vector.tensor_copy(hraw[:, m, :], pf[:])
                    yield

            def normalize(ps_o, h, ch):
                sl = slice(ch * 512, (ch + 1) * 512)
                dn = rpool.tile([1, 512], FP32, name="dn", tag="dn")
                nc.vector.tensor_copy(dn[:], ps_o[D : D + 1, :])
                t1 = rpool.tile([D, 512], FP32, name="t1", tag="t1")
                nc.vector.tensor_mul(t1[:], ps_o[0:D, :], gTh[0:D, h, sl])
                rdb = rpool.tile([D, 512], FP32, name="rdb", tag="rdb")
                nc.gpsimd.partition_broadcast(rdb[:], dn[:])
                nc.vector.reciprocal_approx_fast(rdb[:], rdb[:])
                kt, po = h // 2, (h % 2) * D
                nc.vector.tensor_mul(ogT[kt][po : po + D, sl], t1[:], rdb[:])

            # Flat lag-pipelined loop: scores/exp run LAG tiles ahead of
            # attn@v so the ACT exp stream (the pacer) never stalls on the
            # psO accumulator handoff between head pairs.
            from collections import deque

            seq = [(ch, pr, t) for ch in range(NCH_OWN)
                   for pr in range(CT) for t in range(NT)]
            pending = deque((seq[i], E0.pop(i)) for i in range(N_EARLY))
            ps_now = {}

            def emit_attnv(entry, Et):
                ch, pr, t = entry
                hA, hB = 2 * pr, 2 * pr + 1
                if t == 0:
                    ps_now[0] = psO.tile([P, 512], FP32,
                                         name=f"psoA{ch}{pr}", tag="po")
                    ps_now[1] = psO.tile([P, 512], FP32,
                                         name=f"psoB{ch}{pr}", tag="po")
                nc.tensor.matmul(
                    ps_now[0][:],
                    v_aug[t][:, hA * (D + 1) : hA * (D + 1) + P],
                    Et[:, 0:512],
                    start=(t == 0), stop=(t == NT - 1),
                )
                nc.tensor.matmul(
                    ps_now[1][:],
                    v_aug[t][:, hB * (D + 1) : hB * (D + 1) + P],
                    Et[:, 512:1024],
                    start=(t == 0), stop=(t == NT - 1),
                )
                if t == NT - 1:
                    normalize(ps_now[0], hA, ch)
                    normalize(ps_now[1], hB, ch)

            fill_gen = None
            for entry in seq[N_EARLY:]:
                ch, pr, t = entry
                pending.append((entry, scores_exp(pr, t, ch)))
                e2, Et = pending.popleft()
                emit_attnv(e2, Et)
                if e2 == (0, CT - 1, NT - 1):
                    fill_gen = ch0_mlp_gen()
                if fill_gen is not None:
                    next(fill_gen, None)
            while pending:
                e2, Et = pending.popleft()
                emit_attnv(e2, Et)
            if fill_gen is not None:
                for _ in fill_gen:
                    pass

            es_att.close()  # epool, rpool freed before the MLP tail

            # ---------- tail: ch1 proj/LN3, gelus, fc2s ----------
            with (
                tc.tile_pool(name="hTp", bufs=DFT) as hT_pool,
                tc.tile_pool(name="opool", bufs=4) as opool,
            ):
                hT = [hT_pool.tile([P, TOK_OWN], BF16, name=f"hT{i}", tag="hTt")
                      for i in range(DFT)]
                xm1 = []
                for i in range(4):
                    proj_tile(4 + i, xm1)
                ln_group(xm1, "x31", ("dram", z3_dram_g), 4, lnexp=True)
                transpose_in_z3(1)

                # single gelu-table switch; fc1-ch1 matmuls (PE) overlap the
                # gelu-paced ch0 fc2 stream (ACT runs 2 gelus per m)
                accs = [psO.tile([P, C], FP32, name=f"acc0{i}", tag="po")
                        for i in range(4)]
                for m in range(DFT):
                    pf = psS.tile([P, 512], FP32, name=f"pf1_{m}", tag="sc")
                    for k in range(CT):
                        nc.tensor.matmul(
                            pf[:], wfc1_sb[:, k, m * P : (m + 1) * P],
                            z3T[:, k, 512:1024],
                            start=(k == 0), stop=(k == CT - 1),
                        )
                    nc.scalar.activation(hT[m][:, 0:512], hraw[:, m, :],
                                         Act.Gelu)
                    nc.scalar.activation(hT[m][:, 512:1024], pf[:], Act.Gelu)
                    for i in range(4):
                        nc.tensor.matmul(
                            accs[i][:], hT[m][:, i * P : (i + 1) * P],
                            wfc2_sb[:, m, :],
                            start=(m == 0), stop=(m == DFT - 1),
                        )
                for i in range(4):
                    ot = opool.tile([P, C], FP32, name="ot", tag="ot")
                    nc.vector.tensor_add(ot[:], xm0[i][:], accs[i][:])
                    nc.sync.dma_start(out_r[i], ot[:])

                accs1 = [psO.tile([P, C], FP32, name=f"acc1{i}", tag="po")
                         for i in range(4)]
                for m in range(DFT):
                    for i in range(4):
                        t = 4 + i
                        nc.tensor.matmul(
                            accs1[i][:], hT[m][:, t * P : (t + 1) * P],
                            wfc2_sb[:, m, :],
                            start=(m == 0), stop=(m == DFT - 1),
                        )
                for i in range(4):
                    ot = opool.tile([P, C], FP32, name="ot", tag="ot")
                    nc.vector.tensor_add(ot[:], xm1[i][:], accs1[i][:])
                    nc.sync.dma_start(out_r[4 + i], ot[:])


def _preprocess(inputs):
    """Fold LN affine + attention scale into weights (host-side, weight-only)."""
    f32 = np.float32
    ln1_w, ln1_b = f32(inputs["ln1_w"]), f32(inputs["ln1_b"])
    ln2_w, ln2_b = f32(inputs["ln2_w"]), f32(inputs["ln2_b"])
    ln3_w, ln3_b = f32(inputs["ln3_w"]), f32(inputs["ln3_b"])
    w_qkv = f32(inputs["w_qkv"]).copy()
    w_s = f32(inputs["w_s"])
    w_gate = f32(inputs["w_gate"])
    w_proj = f32(inputs["w_proj"])
    w_fc1 = f32(inputs["w_fc1"])
    w_fc2 = f32(inputs["w_fc2"])

    scale = D ** -0.5
    wqkv_eff = ln1_w[:, None] * w_qkv
    wqkv_eff[:, 0:C] *= scale
    b_qkv = ln1_b @ w_qkv
    b_qkv[0:C] *= scale
    ws_eff = ln2_w[:, None] * w_s
    b_s = ln2_b @ w_s
    wgate_eff = ln1_w[:, None] * w_gate
    b_gate = ln1_b @ w_gate
    wfc1_eff = ln3_w[:, None] * w_fc1
    b_fc1 = ln3_b @ w_fc1 + f32(inputs["b_fc1"])

    for name, bias in [
        ("b_qkv", b_qkv), ("b_s", b_s), ("b_gate", b_gate), ("b_fc1", b_fc1),
        ("b_proj", f32(inputs["b_proj"])), ("b_fc2", f32(inputs["b_fc2"])),
    ]:
        assert np.all(bias == 0.0), f"nonzero bias {name} unsupported by this kernel"

    bf16 = ml_dtypes.bfloat16
    return {
        "wqkv": np.ascontiguousarray(wqkv_eff, dtype=bf16),
        "ws": np.ascontiguousarray(ws_eff, dtype=bf16),
        "wgate": np.ascontiguousarray(wgate_eff, dtype=bf16),
        "wproj": np.ascontiguousarray(w_proj, dtype=bf16),
        "wfc1": np.ascontiguousarray(wfc1_eff, dtype=bf16),
        "wfc2": np.ascontiguousarray(w_fc2, dtype=bf16),
    }


def kernel(**inputs):
    from concourse import bass_utils

    if "nc" not in _CACHE:
        _CACHE["nc"] = _build_nc()
    nc = _CACHE["nc"]

    w = _preprocess(inputs)
    x = np.asarray(inputs["x"], dtype=np.float32)
    e = np.asarray(inputs["e"], dtype=np.float32)

    in_maps = []
    for c in range(N_CORES):
        b, half = c // 2, c % 2
        if half == 0:
            xb, eb = x[b], e[b]
        else:
            xb = np.concatenate([x[b, TOK_OWN:], x[b, :TOK_OWN]], axis=0)
            eb = np.concatenate([e[b, TOK_OWN:], e[b, :TOK_OWN]], axis=0)
        in_maps.append({
            "xb": np.ascontiguousarray(xb),
            "eb": np.ascontiguousarray(eb),
            **w,
        })

    res = bass_utils.run_bass_kernel_spmd(
        nc, in_maps, core_ids=list(range(N_CORES)),
        trace=_CACHE.get("trace", False),
    )
    _CACHE["last_result"] = res

    out = np.empty((B, N, C), dtype=np.float32)
    for c in range(N_CORES):
        b, half = c // 2, c % 2
        out[b, half * TOK_OWN : (half + 1) * TOK_OWN] = res.results[c]["out"]
    return out



# revision 50
# speedup vs baseline: 1.0066x; 1.0066x over previous
"""Trainium2 Bass kernel for nn_Block_48610439856264 (DiT-style transformer block).

B=4, N=2048, C=512, H=8 heads, D=64, d_ff=2048, fp32 I/O.

Sharding: 8 cores = 4 batches x 2 token-halves. Each core receives the full
batch (own token half first) so k/v/s are computed locally over all 2048
tokens (duplicated across the 2 cores of a batch; no collectives), while
q/gate/proj/MLP/output cover only the core's own 1024 tokens.

Device pipeline per core:
  LN1(x), LN2(e) token-major (bn_stats; batched group DMAs since descriptor
  issue on SP is serial) -> bf16 -> xbar transpose to c-major (zx) / DRAM
  roundtrip transpose (ze)
  kk^T = w_k^T zx^T + w_s^T ze^T (PSUM-accumulated), q^T, gate^T (c-major);
  e-independent projections are ordered ahead of kk so the in-order PE never
  head-of-line blocks on the ze roundtrip
  v token-major (lhsT = zx^T blocks), augmented with a ones column per head
  attention runs chunk-major over two 512-token nq chunks; per (pair, tile,
  chunk): one 2-bank PSUM scores tile [A|B], ONE exp on ACT -> E bf16;
  attn@v accumulates per-chunk 1-bank tiles; lag deque keeps exp (the pacer)
  decoupled from attn@v
  chunk-0 proj + LN3 (ln/exp rstd, DRAM-transposed z3) + raw fc1 are emitted
  one unit per attention entry underneath chunk-1's exp-bound stream; MLP
  weights and the x residual reload are prefetched under attention
  tail: chunk-1 proj/LN3, single gelu-table switch, gelu+fc2 interleaved
"""

import numpy as np
import ml_dtypes

N_CORES = 8
B, N, C = 4, 2048, 512
H, D = 8, 64
DFF = 4 * C
P = 128
NT = N // P          # 16 full-token tiles
NTO = NT // 2        # 8 own-token tiles
CT = C // P          # 4 channel tiles
DFT = DFF // P       # 16 d_ff tiles
TOK_OWN = N // 2     # 1024
EPS = 1e-5
NCH_FULL = N // 512      # 4 chunks of 512 tokens
NCH_OWN = TOK_OWN // 512  # 2

_CACHE = {}


def _build_nc():
    import concourse.bacc as bacc
    import concourse.mybir as mybir
    import concourse.tile as tile

    FP32 = mybir.dt.float32
    BF16 = mybir.dt.bfloat16

    nc = bacc.Bacc("TRN2", num_devices=N_CORES)

    # ---- DRAM I/O ----
    xb_d = nc.dram_tensor("xb", [N, C], FP32, kind="ExternalInput").ap()
    eb_d = nc.dram_tensor("eb", [N, C], FP32, kind="ExternalInput").ap()
    wqkv_d = nc.dram_tensor("wqkv", [C, 3 * C], BF16, kind="ExternalInput").ap()
    ws_d = nc.dram_tensor("ws", [C, C], BF16, kind="ExternalInput").ap()
    wgate_d = nc.dram_tensor("wgate", [C, C], BF16, kind="ExternalInput").ap()
    wproj_d = nc.dram_tensor("wproj", [C, C], BF16, kind="ExternalInput").ap()
    wfc1_d = nc.dram_tensor("wfc1", [C, DFF], BF16, kind="ExternalInput").ap()
    wfc2_d = nc.dram_tensor("wfc2", [DFF, C], BF16, kind="ExternalInput").ap()
    out_d = nc.dram_tensor("out", [TOK_OWN, C], FP32, kind="ExternalOutput").ap()

    with tile.TileContext(nc) as tc:
        _build_body(nc, tc, mybir,
                    xb_d, eb_d, out_d,
                    wqkv_d, ws_d, wgate_d, wproj_d, wfc1_d, wfc2_d)

    nc.compile()
    return nc


def _build_body(nc, tc, mybir,
                xb_r_, eb_r_, out_d,
                wqkv_d, ws_d, wgate_d, wproj_d, wfc1_d, wfc2_d):
    from contextlib import ExitStack
    from concourse.masks import make_identity

    FP32 = mybir.dt.float32
    BF16 = mybir.dt.bfloat16
    Act = mybir.ActivationFunctionType
    Alu = mybir.AluOpType

    xb_r = xb_r_.rearrange("(t p) c -> t p c", p=P)
    eb_r = eb_r_.rearrange("(t p) c -> t p c", p=P)
    out_r = out_d.rearrange("(t p) c -> t p c", p=P)

    L0 = ExitStack()
    with L0:
        consts = L0.enter_context(tc.tile_pool(name="consts", bufs=1))
        stat_pool = L0.enter_context(tc.tile_pool(name="stats", bufs=6))
        z_pool = L0.enter_context(tc.tile_pool(name="zp", bufs=2))
        zg_pool = L0.enter_context(tc.tile_pool(name="zg", bufs=1))
        es_x = ExitStack()  # x own-half tiles: freed after LN1, reloaded at proj
        # long-lived attention inputs/outputs
        kkT = [L0.enter_context(tc.tile_pool(name=f"kkT{i}", bufs=1))
               .tile([P, N], BF16, name=f"kkTt{i}") for i in range(CT)]
        qT = [L0.enter_context(tc.tile_pool(name=f"qT{i}", bufs=1))
              .tile([P, TOK_OWN], BF16, name=f"qTt{i}") for i in range(CT)]
        gTh = L0.enter_context(tc.tile_pool(name="gTh", bufs=1)) \
            .tile([D, H, TOK_OWN], BF16, name="gTht")
        vpool = L0.enter_context(tc.tile_pool(name="vp", bufs=NT))
        VPAD = (H - 1) * (D + 1) + P  # pad: last lhsT window ends at 7*65+128
        v_aug = [vpool.tile([P, VPAD], BF16, name=f"vaug{i}", tag="vaug")
                 for i in range(NT)]
        ogT = [L0.enter_context(tc.tile_pool(name=f"ogT{i}", bufs=1))
               .tile([P, TOK_OWN], BF16, name=f"ogTt{i}") for i in range(CT)]
        dram_p = L0.enter_context(tc.tile_pool(name="zdram", bufs=1, space="DRAM"))
        # scores PSUM on the right side: coexists with psC (left) during the
        # projection phase so pair-0 exp can start early.
        psS = L0.enter_context(
            tc.tile_pool(name="psS", bufs=2, space="PSUM", side="right"))

        eps_sb = consts.tile([P, 1], FP32)
        nc.vector.memset(eps_sb[:], EPS)
        ident = consts.tile([P, P], BF16)
        make_identity(nc, ident[:])

        def ln_group(tiles, mvtag, sink, t0, lnexp=False, copy_eng=None):
            g = len(tiles)
            mv = stat_pool.tile([P, g, 2], FP32, name=f"mv_{mvtag}", tag="mv")
            st6 = stat_pool.tile([P, 6], FP32, name=f"st6_{mvtag}", tag="st6")
            for i, xt in enumerate(tiles):
                nc.vector.bn_stats(st6[:], xt[:])
                nc.vector.bn_aggr(mv[:, i, :], st6[:])
            rstd = stat_pool.tile([P, g], FP32, name=f"rstd_{mvtag}", tag="rstd")
            if lnexp:
                # rstd = exp(-0.5*ln(var+eps)): stays inside the ln/exp ACT
                # table, so LN3 can run mid-exp-stream with no table switch
                lnv = stat_pool.tile([P, g], FP32, name=f"lnv_{mvtag}", tag="sd")
                nc.scalar.activation(lnv[:], mv[:, :, 1], Act.Ln, bias=eps_sb[:])
                nc.scalar.activation(rstd[:], lnv[:], Act.Exp, scale=-0.5)
            else:
                sd = stat_pool.tile([P, g], FP32, name=f"sd_{mvtag}", tag="sd")
                nc.scalar.activation(sd[:], mv[:, :, 1], Act.Sqrt, bias=eps_sb[:])
                nc.vector.reciprocal_approx_fast(rstd[:], sd[:])
            # LN apply runs on ACT (Identity is in every act table):
            # z = (x - mu)*rstd = Identity(x*rstd + (-mu*rstd))
            nmr = stat_pool.tile([P, g], FP32, name=f"nmr_{mvtag}", tag="nmr")
            nc.vector.scalar_tensor_tensor(
                nmr[:], mv[:, :, 0], -1.0, rstd[:], Alu.mult, Alu.mult)
            mode, dest = sink
            if mode == "dram":
                zg = zg_pool.tile([P, g, C], BF16, name=f"zg_{mvtag}", tag="zg")
                for i, xt in enumerate(tiles):
                    nc.scalar.activation(
                        zg[:, i, :], xt[:], Act.Identity,
                        bias=nmr[:, i : i + 1], scale=rstd[:, i : i + 1])
                nc.sync.dma_start(dest[t0 // 4], zg[:])
            else:
                T_all, pspool, pstag = dest
                for i, xt in enumerate(tiles):
                    t = t0 + i
                    zt = z_pool.tile([P, C], BF16, name=f"z_{mvtag}_{i}", tag="z")
                    nc.scalar.activation(
                        zt[:], xt[:], Act.Identity,
                        bias=nmr[:, i : i + 1], scale=rstd[:, i : i + 1])
                    for c in range(CT):
                        pt = pspool.tile([P, P], BF16, name=f"pt{c}", tag=pstag)
                        nc.tensor.transpose(
                            pt[:], zt[:, c * P : (c + 1) * P], ident[:]
                        )
                        if copy_eng is None:
                            nc.vector.tensor_copy(
                                T_all[:, c, t * P : (t + 1) * P], pt[:]
                            )
                        else:
                            copy_eng.copy(
                                T_all[:, c, t * P : (t + 1) * P], pt[:]
                            )

        def transpose_in(zdram, T_tiles, ntok):
            for c in range(CT):
                nc.sync.dma_start(
                    T_tiles[c][:, 0:ntok],
                    zdram[:, c * P : (c + 1) * P],
                    transpose=True,
                )

        es_att = ExitStack()  # epool/rpool: right side, closed before MLP
        L2 = ExitStack()      # left-side projection-phase allocations
        try:
            zxT_p = L2.enter_context(tc.tile_pool(name="zxTp", bufs=1))
            zxT = zxT_p.tile([P, CT, N], BF16, name="zxTall", tag="zxT")
            wB = L2.enter_context(tc.tile_pool(name="wB", bufs=1))
            psC = L2.enter_context(tc.tile_pool(name="psC", bufs=2, space="PSUM"))
            L2a = L2.enter_context(ExitStack())
            zeT_p = L2a.enter_context(tc.tile_pool(name="zeTp", bufs=CT))
            zeT = [zeT_p.tile([P, N], BF16, name=f"zeT{i}", tag="zeT")
                   for i in range(CT)]
            wsp = L2a.enter_context(tc.tile_pool(name="wsp", bufs=1))
            xo_pool = es_x.enter_context(tc.tile_pool(name="xo", bufs=2))

            ze_dram = dram_p.tile([N, C], BF16, name="ze_dram")
            ze_dram_g = ze_dram[:].rearrange("(g t p) c -> g p t c", p=P, t=4)

            epool = es_att.enter_context(
                tc.tile_pool(name="epool", bufs=17, side="right"))
            E0 = {}

            def transpose_in_half(zdram, T_tiles, half):
                for c in range(CT):
                    nc.sync.dma_start(
                        T_tiles[c][:, half * TOK_OWN : (half + 1) * TOK_OWN],
                        zdram[half * TOK_OWN : (half + 1) * TOK_OWN,
                              c * P : (c + 1) * P],
                        transpose=True,
                    )

            # ---------- projections + early pair-0 scores ----------
            def q_proj(m):
                pq = psC.tile([P, TOK_OWN], FP32, name=f"pq{m}", tag="pc")
                for k in range(CT):
                    lw = wqkv_sb[:, k, m * P : (m + 1) * P]
                    for ch in range(NCH_OWN):
                        nc.tensor.matmul(
                            pq[:, ch * 512 : (ch + 1) * 512], lw,
                            zxT[:, k, ch * 512 : (ch + 1) * 512],
                            start=(k == 0), stop=(k == CT - 1),
                        )
                nc.vector.tensor_copy(qT[m][:], pq[:])

            def kk_half(m, half):
                # chunks (2*half, 2*half+1) of kk^T row-tile m
                pc = psC.tile([P, TOK_OWN], FP32, name=f"pc{m}_{half}", tag="pc")
                for k in range(CT):
                    lw = wqkv_sb[:, k, C + m * P : C + (m + 1) * P]
                    for i in range(2):
                        ch = 2 * half + i
                        nc.tensor.matmul(
                            pc[:, i * 512 : (i + 1) * 512], lw,
                            zxT[:, k, ch * 512 : (ch + 1) * 512],
                            start=(k == 0), stop=False,
                        )
                for k in range(CT):
                    lw = ws_sb[:, k, m * P : (m + 1) * P]
                    for i in range(2):
                        ch = 2 * half + i
                        nc.tensor.matmul(
                            pc[:, i * 512 : (i + 1) * 512], lw,
                            zeT[k][:, ch * 512 : (ch + 1) * 512],
                            start=False, stop=(k == CT - 1),
                        )
                nc.vector.tensor_copy(
                    kkT[m][:, half * TOK_OWN : (half + 1) * TOK_OWN], pc[:])

            def scores_exp(pr, t, ch):
                # heads A|B side by side: one 2-bank PSUM tile, one exp
                sc = psS.tile([P, 2, 512], FP32, name="scAB", tag="sc")
                nc.tensor.matmul(
                    sc[:, 0, :],
                    kkT[pr][0:D, t * P : (t + 1) * P],
                    qT[pr][0:D, ch * 512 : (ch + 1) * 512],
                    start=True, stop=True,
                )
                nc.tensor.matmul(
                    sc[:, 1, :],
                    kkT[pr][D : 2 * D, t * P : (t + 1) * P],
                    qT[pr][D : 2 * D, ch * 512 : (ch + 1) * 512],
                    start=True, stop=True,
                )
                Et = epool.tile([P, 2 * 512], BF16, name="Et", tag="E")
                nc.scalar.activation(Et[:], sc[:], Act.Exp)
                return Et

            def v_pair(tp):
                # token tiles 2*tp, 2*tp+1
                pv = psC.tile([P, TOK_OWN], FP32, name=f"pv{tp}", tag="pc")
                for k in range(CT):
                    for i in range(2):
                        t = 2 * tp + i
                        nc.tensor.matmul(
                            pv[:, i * 512 : (i + 1) * 512],
                            zxT[:, k, t * P : (t + 1) * P],
                            wqkv_sb[:, k, 2 * C : 3 * C],
                            start=(k == 0), stop=(k == CT - 1),
                        )
                for i in range(2):
                    t = 2 * tp + i
                    nc.gpsimd.memset(v_aug[t][:, H * (D + 1) : VPAD], 0.0)
                    va = v_aug[t][:, 0 : H * (D + 1)].rearrange(
                        "p (h x) -> p h x", x=D + 1)
                    nc.scalar.copy(
                        va[:, :, 0:D],
                        pv[:, i * 512 : (i + 1) * 512].rearrange(
                            "p (h d) -> p h d", d=D),
                    )
                    nc.gpsimd.memset(va[:, :, D : D + 1], 1.0)

            def gate_proj(m):
                pg = psC.tile([P, TOK_OWN], FP32, name=f"pg{m}", tag="pc")
                for k in range(CT):
                    lw = wgate_sb[:, k, m * P : (m + 1) * P]
                    for ch in range(NCH_OWN):
                        nc.tensor.matmul(
                            pg[:, ch * 512 : (ch + 1) * 512], lw,
                            zxT[:, k, ch * 512 : (ch + 1) * 512],
                            start=(k == 0), stop=(k == CT - 1),
                        )
                nc.scalar.copy(gTh[0:D, 2 * m, :], pg[0:D, :])
                nc.scalar.copy(gTh[0:D, 2 * m + 1, :], pg[D : 2 * D, :])

            with (
                tc.tile_pool(name="xrp", bufs=2) as xr_pool,
                tc.tile_pool(name="ep", bufs=2) as e_pool,
            ):
                # one DMA descriptor per 4-tile group: descriptor issue on the
                # SP sequencer is serial (~0.7us each) and gates startup
                xb_g = xb_r_.rearrange("(g t p) c -> g p t c", p=P, t=4)
                eb_g = eb_r_.rearrange("(g t p) c -> g p t c", p=P, t=4)
                xg_tiles = []
                for g in range(2):
                    xt = xo_pool.tile([P, 4, C], FP32, name=f"xg{g}", tag="xg")
                    nc.sync.dma_start(xt[:], xb_g[g])
                    xg_tiles.append(xt)
                wqkv_sb = wB.tile([P, CT, 3 * C], BF16)
                nc.sync.dma_start(
                    wqkv_sb[:], wqkv_d.rearrange("(k p) n -> p k n", p=P))
                eg_tiles = []
                for g in range(2):
                    et = e_pool.tile([P, 4, C], FP32, name=f"eg{g}", tag="e")
                    nc.sync.dma_start(et[:], eb_g[g])
                    eg_tiles.append(et)
                ws_sb = wsp.tile([P, CT, C], BF16)
                nc.sync.dma_start(
                    ws_sb[:], ws_d.rearrange("(k p) n -> p k n", p=P))
                for g in range(2, 4):
                    xt = xr_pool.tile([P, 4, C], FP32, name=f"xg{g}", tag="xr")
                    nc.sync.dma_start(xt[:], xb_g[g])
                    xg_tiles.append(xt)
                wgate_sb = wB.tile([P, CT, C], BF16)
                nc.sync.dma_start(
                    wgate_sb[:], wgate_d.rearrange("(k p) n -> p k n", p=P))
                for g in range(2, 4):
                    et = e_pool.tile([P, 4, C], FP32, name=f"eg{g}", tag="e")
                    nc.sync.dma_start(et[:], eb_g[g])
                    eg_tiles.append(et)
                x_all = [xg_tiles[t // 4][:, t % 4, :] for t in range(NT)]
                e_groups = [[eg_tiles[g][:, i, :] for i in range(4)]
                            for g in range(4)]
                x_own = x_all[:NTO]

                # x-side first: q/v fill the PE while the e-side LN -> DRAM
                # -> transpose roundtrip completes (PE is in-order, so kk must
                # come after enough e-independent work)
                ln_group(x_all[0:4], "x0", ("pe", (zxT, psC, "pc")), 0)
                ln_group(x_all[4:8], "x1", ("pe", (zxT, psC, "pc")), 4)
                q_proj(0)
                ln_group(e_groups[0], "e0", ("dram", ze_dram_g), 0)
                ln_group(e_groups[1], "e1", ("dram", ze_dram_g), 4)
                transpose_in_half(ze_dram[:], zeT, 0)
                v_pair(0)
                v_pair(1)
                ln_group(x_all[8:12], "x2", ("pe", (zxT, psC, "pc")), 8)
                v_pair(2)
                v_pair(3)
                q_proj(1)
                gate_proj(0)
                gate_proj(1)
                kk_half(0, 0)
                E0[0] = scores_exp(0, 0, 0)
                E0[1] = scores_exp(0, 1, 0)
                kk_half(1, 0)
                E0[2] = scores_exp(0, 2, 0)
                E0[3] = scores_exp(0, 3, 0)
                ln_group(e_groups[2], "e2", ("dram", ze_dram_g), 8)
                kk_half(2, 0)
                E0[4] = scores_exp(0, 4, 0)
                E0[5] = scores_exp(0, 5, 0)
                v_pair(4)
                q_proj(2)
                kk_half(3, 0)
                E0[6] = scores_exp(0, 6, 0)
                E0[7] = scores_exp(0, 7, 0)
                ln_group(x_all[12:16], "x3", ("pe", (zxT, psC, "pc")), 12)
                v_pair(5)
                gate_proj(2)
                ln_group(e_groups[3], "e3", ("dram", ze_dram_g), 12)
                transpose_in_half(ze_dram[:], zeT, 1)

            es_x.close()
            N_EARLY = 15
            q_proj(3)
            for tp in range(6, NT // 2):
                v_pair(tp)
            gate_proj(3)
            kk_half(0, 1)
            for tt in range(8, 15):
                E0[tt] = scores_exp(0, tt, 0)
            kk_half(1, 1)
            kk_half(2, 1)
            kk_half(3, 1)
            L2a.close()  # zeT, ws freed
        finally:
            L2.close()  # zxT, wqkv, wgate, psC freed

        rpool = es_att.enter_context(
            tc.tile_pool(name="rpool", bufs=2, side="right"))
        # prefetch MLP weights under the attention stream (L2 SBUF just freed)
        wE = L0.enter_context(tc.tile_pool(name="wE", bufs=1))
        wproj_sb = wE.tile([P, CT, C], BF16)
        nc.sync.dma_start(wproj_sb[:],
                          wproj_d.rearrange("(k p) n -> p k n", p=P))
        wfc1_sb = wE.tile([P, CT, DFF], BF16)
        nc.sync.dma_start(wfc1_sb[:],
                          wfc1_d.rearrange("(k p) n -> p k n", p=P))
        wfc2_sb = wE.tile([P, DFT, C], BF16)
        nc.sync.dma_start(wfc2_sb[:],
                          wfc2_d.rearrange("(k p) n -> p k n", p=P))

        # ---------- attention + chunk-0 MLP fill + tail ----------
        with (
            tc.tile_pool(name="psO", bufs=4, space="PSUM") as psO,
            tc.tile_pool(name="z3Tp", bufs=1) as z3T_pool,
            tc.tile_pool(name="xm", bufs=NTO) as xm_pool,
            tc.tile_pool(name="xrl", bufs=1) as xrl_pool,
            tc.tile_pool(name="hrawp", bufs=1) as hraw_pool,
        ):
            z3T = z3T_pool.tile([P, CT, TOK_OWN], BF16, name="z3Tall",
                                tag="z3Tt")
            hraw = hraw_pool.tile([P, DFT, 512], BF16, name="hraw")
            z3_dram = dram_p.tile([TOK_OWN, C], BF16, name="z3_dram")
            z3_dram_g = z3_dram[:].rearrange("(g t p) c -> g p t c", p=P, t=4)
            xrl0 = xrl_pool.tile([P, 4, C], FP32, name="xrl0", tag="xrl")
            nc.sync.dma_start(xrl0[:], xb_g[0])
            x_own = [None] * NTO
            x_own[0:4] = [xrl0[:, t, :] for t in range(4)]
            xm0 = []

            def transpose_in_z3(ch):
                for c in range(CT):
                    nc.sync.dma_start(
                        z3T[:, c, ch * 512 : (ch + 1) * 512],
                        z3_dram[ch * 512 : (ch + 1) * 512,
                                c * P : (c + 1) * P],
                        transpose=True,
                    )

            def proj_tile(t, xm_sink):
                pp = psO.tile([P, C], FP32, name=f"pp{t}", tag="po")
                for k in range(CT):
                    nc.tensor.matmul(
                        pp[:], ogT[k][:, t * P : (t + 1) * P],
                        wproj_sb[:, k, :],
                        start=(k == 0), stop=(k == CT - 1),
                    )
                xmt = xm_pool.tile([P, C], FP32, name=f"xm{t}", tag="xm")
                nc.vector.tensor_add(xmt[:], x_own[t][:], pp[:])
                xm_sink.append(xmt)

            def ch0_mlp_gen():
                # chunk-0 proj -> LN3 (DRAM-transposed, ln/exp rstd) -> raw
                # fc1 staged to SBUF; one unit emitted per attention entry
                for i in range(4):
                    proj_tile(i, xm0)
                    yield
                ln_group(xm0, "x30", ("dram", z3_dram_g), 0, lnexp=True)
                yield
                transpose_in_z3(0)
                yield
                xrl1 = xrl_pool.tile([P, 4, C], FP32, name="xrl1", tag="xrl")
                nc.sync.dma_start(xrl1[:], xb_g[1])
                x_own[4:8] = [xrl1[:, t, :] for t in range(4)]
                yield
                for _ in range(5):
                    yield  # attention entries cover the z3 DMA roundtrip
                for m in range(DFT):
                    pf = psO.tile([P, 512], FP32, name=f"pf0_{m}", tag="po")
                    for k in range(CT):
                        nc.tensor.matmul(
                            pf[:], wfc1_sb[:, k, m * P : (m + 1) * P],
                            z3T[:, k, 0:512],
                            start=(k == 0), stop=(k == CT - 1),
                        )
                    nc.vector.tensor_copy(hraw[:, m, :], pf[:])
                    yield

            def normalize(ps_o, h, ch):
                sl = slice(ch * 512, (ch + 1) * 512)
                dn = rpool.tile([1, 512], FP32, name="dn", tag="dn")
                nc.vector.tensor_copy(dn[:], ps_o[D : D + 1, :])
                t1 = rpool.tile([D, 512], FP32, name="t1", tag="t1")
                nc.vector.tensor_mul(t1[:], ps_o[0:D, :], gTh[0:D, h, sl])
                rdb = rpool.tile([D, 512], FP32, name="rdb", tag="rdb")
                nc.gpsimd.partition_broadcast(rdb[:], dn[:])
                nc.vector.reciprocal_approx_fast(rdb[:], rdb[:])
                kt, po = h // 2, (h % 2) * D
                nc.vector.tensor_mul(ogT[kt][po : po + D, sl], t1[:], rdb[:])

            # Flat lag-pipelined loop: scores/exp run LAG tiles ahead of
            # attn@v so the ACT exp stream (the pacer) never stalls on the
            # psO accumulator handoff between head pairs.
            from collections import deque

            seq = [(ch, pr, t) for ch in range(NCH_OWN)
                   for pr in range(CT) for t in range(NT)]
            pending = deque((seq[i], E0.pop(i)) for i in range(N_EARLY))
            ps_now = {}

            def emit_attnv(entry, Et):
                ch, pr, t = entry
                hA, hB = 2 * pr, 2 * pr + 1
                if t == 0:
                    ps_now[0] = psO.tile([P, 512], FP32,
                                         name=f"psoA{ch}{pr}", tag="po")
                    ps_now[1] = psO.tile([P, 512], FP32,
                                         name=f"psoB{ch}{pr}", tag="po")
                nc.tensor.matmul(
                    ps_now[0][:],
                    v_aug[t][:, hA * (D + 1) : hA * (D + 1) + P],
                    Et[:, 0:512],
                    start=(t == 0), stop=(t == NT - 1),
                )
                nc.tensor.matmul(
                    ps_now[1][:],
                    v_aug[t][:, hB * (D + 1) : hB * (D + 1) + P],
                    Et[:, 512:1024],
                    start=(t == 0), stop=(t == NT - 1),
                )
                if t == NT - 1:
                    normalize(ps_now[0], hA, ch)
                    normalize(ps_now[1], hB, ch)

            fill_gen = None
            for entry in seq[N_EARLY:]:
                ch, pr, t = entry
                pending.append((entry, scores_exp(pr, t, ch)))
                e2, Et = pending.popleft()
                emit_attnv(e2, Et)
                if e2 == (0, CT - 1, NT - 1):
                    fill_gen = ch0_mlp_gen()
                if fill_gen is not None:
                    next(fill_gen, None)
            while pending:
                e2, Et = pending.popleft()
                emit_attnv(e2, Et)
            if fill_gen is not None:
                for _ in fill_gen:
                    pass

            es_att.close()  # epool, rpool freed before the MLP tail

            # ---------- tail: ch1 proj/LN3, gelus, fc2s ----------
            with (
                tc.tile_pool(name="hTp", bufs=DFT) as hT_pool,
                tc.tile_pool(name="opool", bufs=4) as opool,
            ):
                hT = [hT_pool.tile([P, TOK_OWN], BF16, name=f"hT{i}", tag="hTt")
                      for i in range(DFT)]
                xm1 = []
                for i in range(4):
                    proj_tile(4 + i, xm1)
                ln_group(xm1, "x31", ("dram", z3_dram_g), 4, lnexp=True)
                transpose_in_z3(1)

                # single gelu-table switch; fc1-ch1 matmuls (PE) overlap the
                # gelu-paced ch0 fc2 stream (ACT runs 2 gelus per m)
                accs = [psO.tile([P, C], FP32, name=f"acc0{i}", tag="po")
                        for i in range(4)]
                # fc1-ch1 lags 4 steps behind the ch0 gelu/fc2 stream so its
                # first matmul never head-of-line blocks on the z3 roundtrip
                for mm in range(DFT + 4):
                    if mm < DFT:
                        m = mm
                        nc.scalar.activation(hT[m][:, 0:512], hraw[:, m, :],
                                             Act.Gelu)
                        for i in range(4):
                            nc.tensor.matmul(
                                accs[i][:], hT[m][:, i * P : (i + 1) * P],
                                wfc2_sb[:, m, :],
                                start=(m == 0), stop=(m == DFT - 1),
                            )
                    if mm >= 4:
                        m = mm - 4
                        pf = psS.tile([P, 512], FP32, name=f"pf1_{m}", tag="sc")
                        for k in range(CT):
                            nc.tensor.matmul(
                                pf[:], wfc1_sb[:, k, m * P : (m + 1) * P],
                                z3T[:, k, 512:1024],
                                start=(k == 0), stop=(k == CT - 1),
                            )
                        nc.scalar.activation(hT[m][:, 512:1024], pf[:],
                                             Act.Gelu)
                for i in range(4):
                    ot = opool.tile([P, C], FP32, name="ot", tag="ot")
                    nc.vector.tensor_add(ot[:], xm0[i][:], accs[i][:])
                    nc.sync.dma_start(out_r[i], ot[:])

                accs1 = [psO.tile([P, C], FP32, name=f"acc1{i}", tag="po")
                         for i in range(4)]
                for m in range(DFT):
                    for i in range(4):
                        t = 4 + i
                        nc.tensor.matmul(
                            accs1[i][:], hT[m][:, t * P : (t + 1) * P],
                            wfc2_sb[:, m, :],
                            start=(m == 0), stop=(m == DFT - 1),
                        )
                for i in range(4):
                    ot = opool.tile([P, C], FP32, name="ot", tag="ot")
                    nc.vector.tensor_add(ot[:], xm1[i][:], accs1[i][:])
                    nc.sync.dma_start(out_r[4 + i], ot[:])


def _preprocess(inputs):
    """Fold LN affine + attention scale into weights (host-side, weight-only)."""
    f32 = np.float32
    ln1_w, ln1_b = f32(inputs["ln1_w"]), f32(inputs["ln1_b"])
    ln2_w, ln2_b = f32(inputs["ln2_w"]), f32(inputs["ln2_b"])
    ln3_w, ln3_b = f32(inputs["ln3_w"]), f32(inputs["ln3_b"])
    w_qkv = f32(inputs["w_qkv"]).copy()
    w_s = f32(inputs["w_s"])
    w_gate = f32(inputs["w_gate"])
    w_proj = f32(inputs["w_proj"])
    w_fc1 = f32(inputs["w_fc1"])
    w_fc2 = f32(inputs["w_fc2"])

    scale = D ** -0.5
    wqkv_eff = ln1_w[:, None] * w_qkv
    wqkv_eff[:, 0:C] *= scale
    b_qkv = ln1_b @ w_qkv
    b_qkv[0:C] *= scale
    ws_eff = ln2_w[:, None] * w_s
    b_s = ln2_b @ w_s
    wgate_eff = ln1_w[:, None] * w_gate
    b_gate = ln1_b @ w_gate
    wfc1_eff = ln3_w[:, None] * w_fc1
    b_fc1 = ln3_b @ w_fc1 + f32(inputs["b_fc1"])

    for name, bias in [
        ("b_qkv", b_qkv), ("b_s", b_s), ("b_gate", b_gate), ("b_fc1", b_fc1),
        ("b_proj", f32(inputs["b_proj"])), ("b_fc2", f32(inputs["b_fc2"])),
    ]:
        assert np.all(bias == 0.0), f"nonzero bias {name} unsupported by this kernel"

    bf16 = ml_dtypes.bfloat16
    return {
        "wqkv": np.ascontiguousarray(wqkv_eff, dtype=bf16),
        "ws": np.ascontiguousarray(ws_eff, dtype=bf16),
        "wgate": np.ascontiguousarray(wgate_eff, dtype=bf16),
        "wproj": np.ascontiguousarray(w_proj, dtype=bf16),
        "wfc1": np.ascontiguousarray(wfc1_eff, dtype=bf16),
        "wfc2": np.ascontiguousarray(w_fc2, dtype=bf16),
    }


def kernel(**inputs):
    from concourse import bass_utils

    if "nc" not in _CACHE:
        _CACHE["nc"] = _build_nc()
    nc = _CACHE["nc"]

    w = _preprocess(inputs)
    x = np.asarray(inputs["x"], dtype=np.float32)
    e = np.asarray(inputs["e"], dtype=np.float32)

    in_maps = []
    for c in range(N_CORES):
        b, half = c // 2, c % 2
        if half == 0:
            xb, eb = x[b], e[b]
        else:
            xb = np.concatenate([x[b, TOK_OWN:], x[b, :TOK_OWN]], axis=0)
            eb = np.concatenate([e[b, TOK_OWN:], e[b, :TOK_OWN]], axis=0)
        in_maps.append({
            "xb": np.ascontiguousarray(xb),
            "eb": np.ascontiguousarray(eb),
            **w,
        })

    res = bass_utils.run_bass_kernel_spmd(
        nc, in_maps, core_ids=list(range(N_CORES)),
        trace=_CACHE.get("trace", False),
    )
    _CACHE["last_result"] = res

    out = np.empty((B, N, C), dtype=np.float32)
    for c in range(N_CORES):
        b, half = c // 2, c % 2
        out[b, half * TOK_OWN : (half + 1) * TOK_OWN] = res.results[c]["out"]
    return out

